# revision 40
# baseline (speedup 1.0000x reference)
"""Sliding-window causal self-attention (B=2, T=2048, C=1024, H=16, Dh=64,
window=256) + QKV/out projections, sharded over 8 NeuronCores as
data-parallel over B (2) x tensor-parallel over head groups (4 heads/core).

Layout strategy ("sT scheme"): scores are computed TRANSPOSED
(sT[k, q] = khT^T @ qhT) so the exp() activation writes P^T straight to
SBUF. The band mask is a post-exp 0/1 multiply on bf16 SBUF data; score
blocks are stored [mid, first, diag] so the two masked blocks are one
contiguous [128, 256] region and the mask is a single DVE multiply per
head pair. Row sums come free from a ones-column appended to each head's
V tile; the 4 per-head PV outputs share one PSUM bank so normalization
is one batched reciprocal + one broadcast multiply per query tile. The
attention output is transposed back with PE identity matmuls before the
out-proj.

DMA strategy: all DRAM tensors are pre-arranged on the host to exactly
match their SBUF destination layout, so every load is a single dma_start
with one max-length descriptor per partition (sequencer issue cost and
queue descriptor count are the real DMA bottlenecks, not bytes). The
first QKV chain's inputs (w q-blocks, x split 0) are the first issues on
the two HWDGE queues; everything else streams behind in need-order.
Output tiles are stored as [128, 2048] pairs (8 stores total).

Scheduling: software-pipelined per query tile as in the previous
revision: QKV token-splits are emitted ahead of their attention
consumers; RoPE runs on DVE (splits 0-1) and the otherwise-idle GpSimd
engine (splits 2-3); q-side repack DMAs issue from the SP queue and
k-side from the ACT queue.
"""

import math

import numpy as np

B = 2
T = 2048
C = 1024
H = 16
DH = 64
WINDOW = 256
HEADS_PER_CORE = 4
N_CORES = 8
QT = T // 128  # 16 query tiles of 128
FQ = HEADS_PER_CORE * DH  # 256 local features
VW = DH + 1  # per-head v columns incl the fused ones column
VROW = HEADS_PER_CORE * VW  # 260 v columns per key tile

_PROGRAM = None  # compile once per process


def _emit(nc, tc, aps, ctx):
    from concourse import mybir

    f32 = mybir.dt.float32
    bf16 = mybir.dt.bfloat16
    Exp = mybir.ActivationFunctionType.Exp
    Mult = mybir.AluOpType.mult

    xT, wT, woT, cos4, sin4, amask01, ident, y = (
        aps["xT"], aps["wT"], aps["woT"], aps["cos4"], aps["sin4"],
        aps["amask01"], aps["ident"], aps["y"],
    )

    consts = ctx.enter_context(tc.tile_pool(name="consts", bufs=1))
    stage = ctx.enter_context(tc.tile_pool(name="stage", bufs=1))
    pre = ctx.enter_context(tc.tile_pool(name="pre", bufs=8))
    tmp = ctx.enter_context(tc.tile_pool(name="tmp", bufs=3))
    work = ctx.enter_context(tc.tile_pool(name="work", bufs=6))
    osbp = ctx.enter_context(tc.tile_pool(name="osbp", bufs=2))
    asbp = ctx.enter_context(tc.tile_pool(name="asbp", bufs=2))
    ysbp = ctx.enter_context(tc.tile_pool(name="ysbp", bufs=3))
    small = ctx.enter_context(tc.tile_pool(name="small", bufs=4))
    pmm = ctx.enter_context(tc.tile_pool(name="pmm", bufs=2, space="PSUM"))
    pout = ctx.enter_context(tc.tile_pool(name="pout", bufs=2, space="PSUM"))
    ps = ctx.enter_context(tc.tile_pool(name="ps", bufs=2, space="PSUM"))
    po = ctx.enter_context(tc.tile_pool(name="po", bufs=2, space="PSUM"))

    # ---- resident inputs ----
    # x in [C-chunk partition, (split, kc, 512 tokens)] — matches DRAM
    xT_sb = consts.tile([128, 4 * 8 * 512], bf16, tag="xT")
    # w in [C-chunk partition, (blk, kc, 128 rows)], blk = q1 q2 k1 k2 v0 v1
    wT_sb = consts.tile([128, 6 * 8 * 128], bf16, tag="wT")
    woT_sb = consts.tile([128, 2 * C], bf16, tag="woT")
    cos_sb = consts.tile([128, T], bf16, tag="cos")
    sin_sb = consts.tile([128, T], bf16, tag="sin")
    amask_sb = consts.tile([128, 256], bf16, tag="amask")
    id_sb = consts.tile([128, 128], bf16, tag="ident")

    # SP queue: first-chain inputs first, then stream in need-order.
    # Each x split is ONE dma so a QKV chain waits once and then runs all
    # 8 accumulation steps gapless (mid-chain stalls also reset the PE
    # clock ramp).
    nc.sync.dma_start(out=wT_sb[:, 0:2048], in_=wT[:, 0:2048])  # q1 q2
    for s in range(2):
        nc.sync.dma_start(out=xT_sb[:, s * 4096:(s + 1) * 4096],
                          in_=xT[:, s * 4096:(s + 1) * 4096])
    nc.sync.dma_start(out=cos_sb, in_=cos4)
    nc.sync.dma_start(out=sin_sb, in_=sin4)
    # ACT queue: k/v weights, mask, x splits 2-3, transpose id, out-proj w.
    # amask goes first so the big w load doesn't race x split 0 for HBM.
    nc.scalar.dma_start(out=amask_sb, in_=amask01)
    nc.scalar.dma_start(out=wT_sb[:, 2048:6144], in_=wT[:, 2048:6144])
    for s in range(2, 4):
        nc.scalar.dma_start(out=xT_sb[:, s * 4096:(s + 1) * 4096],
                            in_=xT[:, s * 4096:(s + 1) * 4096])
    nc.scalar.dma_start(out=id_sb, in_=ident)
    nc.scalar.dma_start(out=woT_sb, in_=woT)

    # ---- persistent intermediates ----
    # rotated q/k blocks [q_x1, q_x2, k_x1, k_x2], each [128=(4h x 32d), T]
    rot = [stage.tile([128, T], bf16, tag=f"rot{i}", name=f"rot{i}")
           for i in range(4)]
    qhT = stage.tile([64, HEADS_PER_CORE * T], bf16, tag="qhT")
    khT = stage.tile([64, HEADS_PER_CORE * T], bf16, tag="khT")
    # v in [k-token-part, (kt, head, 65)] layout; col 64 of each head = ones
    v_sb = stage.tile([128, QT * VROW], bf16, tag="v")
    nc.gpsimd.memset(
        v_sb.rearrange("p (g c) -> p g c", c=VW)[:, :, DH:DH + 1], 1.0)

    pres = {}  # split -> [pre tiles]

    def qkv_half(split, pair, alt_pre=None):
        """QKV projection matmuls + PSUM->SBUF casts for the q or k blocks
        of one token slice. alt_pre routes the odd-block cast to another
        engine to avoid piling copies onto ACT ahead of critical exps."""
        ptiles = pres.setdefault(split, [])
        for blk in (2 * pair, 2 * pair + 1):  # q_x1 q_x2 | k_x1 k_x2
            acc = pmm.tile([128, 512], f32, tag="mm")
            for kc in range(8):
                nc.tensor.matmul(
                    acc,
                    lhsT=wT_sb[:, blk * 1024 + kc * 128:blk * 1024 + (kc + 1) * 128],
                    rhs=xT_sb[:, split * 4096 + kc * 512:split * 4096 + (kc + 1) * 512],
                    start=(kc == 0),
                    stop=(kc == 7),
                )
            pblk = pre.tile([128, 512], bf16, tag="pre", name=f"pre{split}{blk}")
            if alt_pre is not None and blk % 2 == 1:
                alt_pre.tensor_copy(pblk, acc)
            else:
                nc.scalar.copy(pblk, acc)
            ptiles.append(pblk)

    rope_tmp = {}

    def rope_chunk(split, pair, eng, chunk):
        """Two of the six rope ops; chunks can be spread across iterations
        so a DVE rope burst never delays the latency-critical mask ops."""
        tsl = slice(split * 512, (split + 1) * 512)
        x1, x2 = pres[split][2 * pair], pres[split][2 * pair + 1]
        r1, r2 = rot[2 * pair][:, tsl], rot[2 * pair + 1][:, tsl]
        if chunk == 0:
            rope_tmp[(split, pair)] = [
                tmp.tile([128, 512], bf16, tag=t, name=t)
                for t in ("t1", "t2", "t3", "t4")]
        t1, t2, t3, t4 = rope_tmp[(split, pair)]
        if chunk == 0:
            eng.tensor_mul(t1, x1, cos_sb[:, tsl])
            eng.tensor_mul(t2, x2, sin_sb[:, tsl])
        elif chunk == 1:
            eng.tensor_sub(r1, t1, t2)
            eng.tensor_mul(t3, x2, cos_sb[:, tsl])
        else:
            eng.tensor_mul(t4, x1, sin_sb[:, tsl])
            eng.tensor_add(r2, t3, t4)

    def rope(split, pair, eng):
        """rot1 = x1*cos - x2*sin ; rot2 = x2*cos + x1*sin for one pair."""
        for chunk in range(3):
            rope_chunk(split, pair, eng, chunk)

    def repack(t0, tlen):
        """Repack a token range of rot into head-major qhT/khT; q-side DMAs
        issue from the SP queue, k-side from GpSimd (SWDGE) so the rope-gated
        issues never block the ACT compute stream."""
        tsl = slice(t0, t0 + tlen)
        for hl in range(HEADS_PER_CORE):
            d0 = hl * T + t0
            for half in range(2):
                nc.sync.dma_start(
                    out=qhT[half * 32:(half + 1) * 32, d0:d0 + tlen],
                    in_=rot[half][hl * 32:(hl + 1) * 32, tsl],
                )
                nc.gpsimd.dma_start(
                    out=khT[half * 32:(half + 1) * 32, d0:d0 + tlen],
                    in_=rot[2 + half][hl * 32:(hl + 1) * 32, tsl],
                )

    # ---- software-pipelined attention ----
    st = {}  # qt -> {p:{hl: tile}, osb:, asb:}
    ysbs = {}  # qt-pair -> ysb tile

    def wincfg(qt):
        nkt = min(qt + 1, 3)
        return nkt, max(qt - 2, 0)

    def slot_of(qt, a):
        # score block column slots: [mid, first, diag] for qt>=2 so the two
        # masked blocks (first, diag) are one contiguous [128, 256] region
        if qt < 2:
            return a
        return (1, 0, 2)[a]

    def emit_v(qt):
        """V tile for qt in [k-part, (head, 65)] layout (ones col fused)."""
        s, off = qt // 4, (qt % 4) * 128
        acc = pmm.tile([128, FQ], f32, tag="mm")
        for kc in range(8):
            nc.tensor.matmul(
                acc,
                lhsT=xT_sb[:, s * 4096 + kc * 512 + off:s * 4096 + kc * 512 + off + 128],
                rhs=wT_sb.rearrange("p (b k j) -> p b k j", b=6, k=8)[:, 4:6, kc, :],
                start=(kc == 0),
                stop=(kc == 7),
            )
        nc.scalar.copy(
            v_sb[:, qt * VROW:(qt + 1) * VROW]
            .rearrange("p (h c) -> p h c", h=HEADS_PER_CORE)[:, :, 0:DH],
            acc.rearrange("p (h d) -> p h d", h=HEADS_PER_CORE),
        )

    def emit_scores(qt, pair):
        """Transposed scores + exp + band mask for one pair of heads.
        P^T for both heads lives in one [128, 768] tile so the mask is a
        single strided DVE multiply."""
        nkt, kt0 = wincfg(qt)
        ss = st.setdefault(qt, {"p": {}})
        pp = work.tile([128, 768], bf16, tag="p")
        for i, hl in enumerate((2 * pair, 2 * pair + 1)):
            s = ps.tile([128, 384], f32, tag="s")
            for a in range(nkt):
                kt = kt0 + a
                nc.tensor.matmul(
                    s[:, slot_of(qt, a) * 128:(slot_of(qt, a) + 1) * 128],
                    lhsT=khT[:, hl * T + kt * 128:hl * T + (kt + 1) * 128],
                    rhs=qhT[:, hl * T + qt * 128:hl * T + (qt + 1) * 128],
                    start=True,
                    stop=True,
                )
            w = 128 * nkt
            nc.scalar.activation(pp[:, i * 384:i * 384 + w], s[:, :w], Exp)
            ss["p"][hl] = pp[:, i * 384:(i + 1) * 384]
        ppv = pp.rearrange("p (i c) -> p i c", i=2)
        if qt >= 2:  # mask blocks [first|diag] at cols 128:384 of each half
            nc.vector.tensor_tensor(
                ppv[:, :, 128:384], ppv[:, :, 128:384],
                amask_sb.rearrange("p (one c) -> p one c", one=1)
                .broadcast_to([128, 2, 256]),
                Mult)
        else:  # single diag block: slot nkt-1
            c0 = (nkt - 1) * 128
            nc.vector.tensor_tensor(
                ppv[:, :, c0:c0 + 128], ppv[:, :, c0:c0 + 128],
                amask_sb.rearrange("p (one c) -> p one c", one=1)
                [:, :, 128:256].broadcast_to([128, 2, 128]),
                Mult)

    def emit_ot(qt):
        """P^T @ [V|1] per head into one shared PSUM bank, then one batched
        reciprocal + one broadcast multiply for the normalization."""
        nkt, kt0 = wincfg(qt)
        ss = st[qt]
        osb = osbp.tile([128, FQ], bf16, tag="osb")
        ss["osb"] = osb
        oall = po.tile([128, VROW], f32, tag="o")
        for hl in range(HEADS_PER_CORE):
            p = ss["p"][hl]
            for a in range(nkt):
                kt = kt0 + a
                sl = slot_of(qt, a)
                nc.tensor.matmul(
                    oall[:, hl * VW:(hl + 1) * VW],
                    lhsT=p[:, sl * 128:(sl + 1) * 128],
                    rhs=v_sb[:, kt * VROW + hl * VW:kt * VROW + (hl + 1) * VW],
                    start=(a == 0),
                    stop=(a == nkt - 1),
                )
        ov = oall.rearrange("p (h c) -> p h c", c=VW)
        rc = small.tile([128, HEADS_PER_CORE], f32, tag="rc")
        rcv = rc.rearrange("p (h one) -> p h one", one=1)
        nc.vector.reciprocal(rcv, ov[:, :, DH:DH + 1])
        nc.vector.tensor_tensor(
            osb.rearrange("p (h d) -> p h d", d=DH),
            ov[:, :, 0:DH],
            rcv.broadcast_to([128, HEADS_PER_CORE, DH]),
            Mult)

    def emit_tp(qt):
        """PE-transpose the attention output to [feature, token]."""
        if qt < 0:
            return
        ss = st[qt]
        t2 = po.tile([128, FQ], bf16, tag="o", name="t2")
        for c in range(2):
            nc.tensor.transpose(
                t2[:, c * 128:(c + 1) * 128],
                ss["osb"][:, c * 128:(c + 1) * 128], id_sb)
        asb = asbp.tile([128, FQ], bf16, tag="asb")
        nc.vector.tensor_copy(asb, t2)
        ss["asb"] = asb

    def emit_outproj(qt):
        """Out-proj for qt, staged through SBUF (cast to bf16); token-tile
        pairs are stored with a single [128, 2048] DMA."""
        if qt < 0:
            return
        asb = st[qt]["asb"]
        single = qt >= QT - 2  # last two tiles stored alone (shorter tail)
        if qt % 2 == 0 or single:
            ysbs[qt // 2] = ysbp.tile([128, 2 * C], bf16, tag="ysb",
                                      name="ysb")
        ysb = ysbs[qt // 2]
        base = 0 if single else (qt % 2) * C
        for nh in range(2):
            acc = pout.tile([128, 512], f32, tag="yp")
            for kc in range(2):
                nc.tensor.matmul(
                    acc,
                    lhsT=asb[:, kc * 128:(kc + 1) * 128],
                    rhs=woT_sb[:, kc * C + nh * 512:kc * C + (nh + 1) * 512],
                    start=(kc == 0),
                    stop=(kc == 1),
                )
            if nh == 0:
                nc.scalar.copy(ysb[:, base:base + 512], acc)
            else:
                nc.vector.tensor_copy(ysb[:, base + 512:base + 1024], acc)
            if single:  # tail tiles: stream each half out as soon as cast
                nc.sync.dma_start(
                    out=y[:, qt * C + nh * 512:qt * C + (nh + 1) * 512],
                    in_=ysb[:, nh * 512:(nh + 1) * 512])
        if single:
            pass
        elif qt % 2 == 1:
            nc.sync.dma_start(
                out=y[:, (qt - 1) * C:(qt + 1) * C], in_=ysb)
        del st[qt]

    def attn_iter(qt):
        # scores/exp/mask for qt+1 are emitted one full iteration before
        # PV(qt+1) consumes them, so PV never waits on the exp->mask chain
        emit_tp(qt - 1)
        if qt + 2 < QT:
            emit_v(qt + 2)
        if qt + 1 < QT:
            emit_scores(qt + 1, 0)
            emit_scores(qt + 1, 1)
        emit_ot(qt)
        emit_outproj(qt - 1)

    # ---- prologue: projections for token half 0, RoPE on idle DVE ----
    qkv_half(0, 0)
    qkv_half(0, 1)
    rope(0, 0, nc.vector)
    rope(0, 1, nc.vector)
    repack(0, 512)  # unblocks qt 0-3 without waiting on split-1 rope
    qkv_half(1, 0)
    qkv_half(1, 1)
    rope(1, 0, nc.vector)
    rope(1, 1, nc.vector)
    repack(512, 512)
    emit_v(0)
    emit_v(1)
    qkv_half(2, 0)  # keeps the PE busy while the repack lands
    qkv_half(2, 1)
    emit_scores(0, 0)
    emit_scores(0, 1)

    # ---- attention pipeline, with split 2/3 projections interleaved ----
    # steady-state RoPE is split between GpSimd and DVE so that both the
    # q-side (rot0/1) and k-side (rot2/3) of each late repack range are
    # ready well before the score matmuls that consume them
    attn_iter(0)
    qkv_half(3, 0, alt_pre=nc.vector)
    rope(2, 1, nc.gpsimd)  # k-side ropes run first on GpSimd
    attn_iter(1)
    qkv_half(3, 1, alt_pre=nc.vector)
    rope_chunk(2, 0, nc.vector, 0)  # q-side ropes on DVE, spread out
    attn_iter(2)
    rope_chunk(2, 0, nc.vector, 1)
    rope(3, 1, nc.gpsimd)
    attn_iter(3)
    rope_chunk(2, 0, nc.vector, 2)
    repack(1024, 512)
    attn_iter(4)
    rope_chunk(3, 0, nc.vector, 0)
    attn_iter(5)
    rope_chunk(3, 0, nc.vector, 1)
    attn_iter(6)
    rope_chunk(3, 0, nc.vector, 2)
    repack(1536, 512)
    for qt in range(7, QT):
        attn_iter(qt)
    emit_tp(QT - 1)
    emit_outproj(QT - 1)


def _build_program():
    import concourse.tile as tile
    from concourse import bacc, mybir

    bf16 = mybir.dt.bfloat16

    nc = bacc.Bacc("TRN2", target_bir_lowering=False, debug=False,
                   num_devices=N_CORES)
    aps = {
        "xT": nc.dram_tensor("xT", [128, 4 * 8 * 512], bf16, kind="ExternalInput").ap(),
        "wT": nc.dram_tensor("wT", [128, 6 * 8 * 128], bf16, kind="ExternalInput").ap(),
        "woT": nc.dram_tensor("woT", [128, 2 * C], bf16, kind="ExternalInput").ap(),
        "cos4": nc.dram_tensor("cos4", [128, T], bf16, kind="ExternalInput").ap(),
        "sin4": nc.dram_tensor("sin4", [128, T], bf16, kind="ExternalInput").ap(),
        "amask01": nc.dram_tensor("amask01", [128, 256], bf16, kind="ExternalInput").ap(),
        "ident": nc.dram_tensor("ident", [128, 128], bf16, kind="ExternalInput").ap(),
        "y": nc.dram_tensor("y", [128, QT * C], bf16, kind="ExternalOutput").ap(),
    }
    from contextlib import ExitStack

    with tile.TileContext(nc) as tc, ExitStack() as ctx:
        _emit(nc, tc, aps, ctx)
    nc.compile()
    return nc


def _get_program():
    global _PROGRAM
    if _PROGRAM is None:
        _PROGRAM = _build_program()
    return _PROGRAM


def _host_inputs(x, w_qkv, w_out):
    import ml_dtypes

    bf16 = ml_dtypes.bfloat16
    x = np.asarray(x, np.float32)
    w_qkv = np.asarray(w_qkv, np.float32)
    w_out = np.asarray(w_out, np.float32)

    wq, wk, wv = w_qkv[0:C], w_qkv[C:2 * C], w_qkv[2 * C:3 * C]
    scale = 1.0 / math.sqrt(DH)

    # RoPE tables (transposed, tiled over the 4 heads of a block)
    inv_freq = 1.0 / (10000.0 ** (np.arange(0, DH, 2, dtype=np.float32) / DH))
    freqs = np.outer(np.arange(T, dtype=np.float32), inv_freq)  # [T, 32]
    cos4 = np.ascontiguousarray(np.tile(np.cos(freqs).T, (4, 1))).astype(bf16)
    sin4 = np.ascontiguousarray(np.tile(np.sin(freqs).T, (4, 1))).astype(bf16)

    # multiplicative 0/1 band masks for TRANSPOSED probabilities pT[k, q]:
    # [block kt=qt-2: allowed qq < kk | block kt=qt: allowed qq >= kk]
    i = np.arange(128)[:, None]  # kk (partitions)
    c = np.arange(128)[None, :]  # qq (free)
    m_first = (c < i).astype(np.float32)
    m_last = (c >= i).astype(np.float32)
    amask01 = np.ascontiguousarray(
        np.concatenate([m_first, m_last], axis=1)).astype(bf16)
    ident = np.eye(128, dtype=np.float32).astype(bf16)

    # x relayout to [C-chunk partition, (split, kc, 512)], matching SBUF
    xT = []
    for b in range(B):
        a = x[b].reshape(4, 512, 8, 128)  # [s, t', kc, p]
        xT.append(np.ascontiguousarray(
            a.transpose(3, 0, 2, 1).reshape(128, 4 * 8 * 512)).astype(bf16))

    in_maps = []
    for core in range(N_CORES):
        b, g = divmod(core, 4)
        hs = range(4 * g, 4 * g + 4)
        rows = []
        for half in range(2):  # q_x1, q_x2
            rows.append(np.concatenate(
                [wq[h * DH + 32 * half:h * DH + 32 * half + 32] for h in hs]) * scale)
        for half in range(2):  # k_x1, k_x2
            rows.append(np.concatenate(
                [wk[h * DH + 32 * half:h * DH + 32 * half + 32] for h in hs]))
        rows.append(wv[g * FQ:(g + 1) * FQ])
        wmat = np.concatenate(rows)  # [768, C]
        # relayout to [C-chunk partition, (blk, kc, 128 rows)]
        wa = wmat.reshape(6, 128, 8, 128)  # [blk, j, kc, p]
        wT = np.ascontiguousarray(
            wa.transpose(3, 0, 2, 1).reshape(128, 6 * 8 * 128)).astype(bf16)
        # w_out columns for this head group, [feat-chunk part, (kc, C)]
        wo = w_out[:, g * FQ:(g + 1) * FQ].T.reshape(2, 128, C)  # [kc, p, e]
        woT = np.ascontiguousarray(
            wo.transpose(1, 0, 2).reshape(128, 2 * C)).astype(bf16)
        in_maps.append({
            "xT": xT[b], "wT": wT, "woT": woT,
            "cos4": cos4, "sin4": sin4, "amask01": amask01, "ident": ident,
        })
    return in_maps


def kernel(x, w_qkv, w_out, _trace=False):
    from concourse import bass_utils

    nc = _get_program()
    in_maps = _host_inputs(x, w_qkv, w_out)
    res = bass_utils.run_bass_kernel_spmd(
        nc, in_maps, core_ids=list(range(N_CORES)), trace=_trace,
    )
    parts = []
    for core in range(N_CORES):
        yv = np.asarray(res.results[core]["y"], dtype=np.float32)
        # [128, (qt, C)] -> [T, C]
        parts.append(yv.reshape(128, QT, C).transpose(1, 0, 2).reshape(T, C))
    out = np.stack([
        parts[0] + parts[1] + parts[2] + parts[3],
        parts[4] + parts[5] + parts[6] + parts[7],
    ])
    if _trace:
        return out, res
    return out


# revision 41
# speedup vs baseline: 1.0178x; 1.0178x over previous
"""Sliding-window causal self-attention (B=2, T=2048, C=1024, H=16, Dh=64,
window=256) + QKV/out projections, sharded over 8 NeuronCores as
data-parallel over B (2) x tensor-parallel over head groups (4 heads/core).

Layout strategy ("sT scheme"): scores are computed TRANSPOSED
(sT[k, q] = khT^T @ qhT) so the exp() activation writes P^T straight to
SBUF. The band mask is a post-exp 0/1 multiply on bf16 SBUF data; score
blocks are stored [mid, first, diag] so the two masked blocks are one
contiguous [128, 256] region and the mask is a single DVE multiply per
head pair. Row sums come free from a ones-column appended to each head's
V tile; the 4 per-head PV outputs share one PSUM bank so normalization
is one batched reciprocal + one broadcast multiply per query tile. The
attention output is transposed back with PE identity matmuls before the
out-proj.

DMA strategy: all DRAM tensors are pre-arranged on the host to exactly
match their SBUF destination layout, so every load is a single dma_start
with one max-length descriptor per partition (sequencer issue cost and
queue descriptor count are the real DMA bottlenecks, not bytes). The
first QKV chain's inputs (w q-blocks, x split 0) are the first issues on
the two HWDGE queues; everything else streams behind in need-order.
Output tiles are stored as [128, 2048] pairs (8 stores total).

Scheduling: software-pipelined per query tile as in the previous
revision: QKV token-splits are emitted ahead of their attention
consumers; RoPE runs on DVE (splits 0-1) and the otherwise-idle GpSimd
engine (splits 2-3); q-side repack DMAs issue from the SP queue and
k-side from the ACT queue.
"""

import math

import numpy as np

B = 2
T = 2048
C = 1024
H = 16
DH = 64
WINDOW = 256
HEADS_PER_CORE = 4
N_CORES = 8
QT = T // 128  # 16 query tiles of 128
FQ = HEADS_PER_CORE * DH  # 256 local features
VW = DH + 1  # per-head v columns incl the fused ones column
VROW = HEADS_PER_CORE * VW  # 260 v columns per key tile

_PROGRAM = None  # compile once per process


def _emit(nc, tc, aps, ctx):
    from concourse import mybir

    f32 = mybir.dt.float32
    bf16 = mybir.dt.bfloat16
    Exp = mybir.ActivationFunctionType.Exp
    Mult = mybir.AluOpType.mult

    xT, wT, woT, cos4, sin4, amask01, ident, y = (
        aps["xT"], aps["wT"], aps["woT"], aps["cos4"], aps["sin4"],
        aps["amask01"], aps["ident"], aps["y"],
    )

    consts = ctx.enter_context(tc.tile_pool(name="consts", bufs=1))
    stage = ctx.enter_context(tc.tile_pool(name="stage", bufs=1))
    pre = ctx.enter_context(tc.tile_pool(name="pre", bufs=8))
    tmp = ctx.enter_context(tc.tile_pool(name="tmp", bufs=3))
    work = ctx.enter_context(tc.tile_pool(name="work", bufs=6))
    osbp = ctx.enter_context(tc.tile_pool(name="osbp", bufs=2))
    asbp = ctx.enter_context(tc.tile_pool(name="asbp", bufs=2))
    ysbp = ctx.enter_context(tc.tile_pool(name="ysbp", bufs=3))
    small = ctx.enter_context(tc.tile_pool(name="small", bufs=4))
    pmm = ctx.enter_context(tc.tile_pool(name="pmm", bufs=2, space="PSUM"))
    pout = ctx.enter_context(tc.tile_pool(name="pout", bufs=2, space="PSUM"))
    ps = ctx.enter_context(tc.tile_pool(name="ps", bufs=2, space="PSUM"))
    po = ctx.enter_context(tc.tile_pool(name="po", bufs=2, space="PSUM"))

    # ---- resident inputs ----
    # x in [C-chunk partition, (split, kc, 512 tokens)] — matches DRAM
    xT_sb = consts.tile([128, 4 * 8 * 512], bf16, tag="xT")
    # w in [C-chunk partition, (blk, kc, 128 rows)], blk = q1 q2 k1 k2 v0 v1
    wT_sb = consts.tile([128, 6 * 8 * 128], bf16, tag="wT")
    woT_sb = consts.tile([128, 2 * C], bf16, tag="woT")
    cos_sb = consts.tile([128, T], bf16, tag="cos")
    sin_sb = consts.tile([128, T], bf16, tag="sin")
    amask_sb = consts.tile([128, 256], bf16, tag="amask")
    id_sb = consts.tile([128, 128], bf16, tag="ident")

    # SP queue: first-chain inputs first, then stream in need-order.
    # Each x split is ONE dma so a QKV chain waits once and then runs all
    # 8 accumulation steps gapless (mid-chain stalls also reset the PE
    # clock ramp).
    nc.sync.dma_start(out=wT_sb[:, 0:2048], in_=wT[:, 0:2048])  # q1 q2
    for s in range(2):
        nc.sync.dma_start(out=xT_sb[:, s * 4096:(s + 1) * 4096],
                          in_=xT[:, s * 4096:(s + 1) * 4096])
    nc.sync.dma_start(out=cos_sb, in_=cos4)
    nc.sync.dma_start(out=sin_sb, in_=sin4)
    # ACT queue: k/v weights, mask, x splits 2-3, transpose id, out-proj w.
    # amask goes first so the big w load doesn't race x split 0 for HBM.
    nc.scalar.dma_start(out=amask_sb, in_=amask01)
    nc.scalar.dma_start(out=wT_sb[:, 2048:6144], in_=wT[:, 2048:6144])
    for s in range(2, 4):
        nc.scalar.dma_start(out=xT_sb[:, s * 4096:(s + 1) * 4096],
                            in_=xT[:, s * 4096:(s + 1) * 4096])
    nc.scalar.dma_start(out=id_sb, in_=ident)
    nc.scalar.dma_start(out=woT_sb, in_=woT)

    # ---- persistent intermediates ----
    # rotated q/k blocks [q_x1, q_x2, k_x1, k_x2], each [128=(4h x 32d), T]
    rot = [stage.tile([128, T], bf16, tag=f"rot{i}", name=f"rot{i}")
           for i in range(4)]
    qhT = stage.tile([64, HEADS_PER_CORE * T], bf16, tag="qhT")
    khT = stage.tile([64, HEADS_PER_CORE * T], bf16, tag="khT")
    # v in [k-token-part, (kt, head, 65)] layout; col 64 of each head = ones
    v_sb = stage.tile([128, QT * VROW], bf16, tag="v")
    nc.gpsimd.memset(
        v_sb.rearrange("p (g c) -> p g c", c=VW)[:, :, DH:DH + 1], 1.0)

    pres = {}  # split -> [pre tiles]

    def qkv_half(split, pair, alt_pre=None):
        """QKV projection matmuls + PSUM->SBUF casts for the q or k blocks
        of one token slice. alt_pre routes the odd-block cast to another
        engine to avoid piling copies onto ACT ahead of critical exps."""
        ptiles = pres.setdefault(split, [])
        for blk in (2 * pair, 2 * pair + 1):  # q_x1 q_x2 | k_x1 k_x2
            acc = pmm.tile([128, 512], f32, tag="mm")
            for kc in range(8):
                nc.tensor.matmul(
                    acc,
                    lhsT=wT_sb[:, blk * 1024 + kc * 128:blk * 1024 + (kc + 1) * 128],
                    rhs=xT_sb[:, split * 4096 + kc * 512:split * 4096 + (kc + 1) * 512],
                    start=(kc == 0),
                    stop=(kc == 7),
                )
            pblk = pre.tile([128, 512], bf16, tag="pre", name=f"pre{split}{blk}")
            if alt_pre is not None and blk % 2 == 1:
                alt_pre.tensor_copy(pblk, acc)
            else:
                nc.scalar.copy(pblk, acc)
            ptiles.append(pblk)

    rope_tmp = {}

    def rope_chunk(split, pair, eng, chunk):
        """Two of the six rope ops; chunks can be spread across iterations
        so a DVE rope burst never delays the latency-critical mask ops."""
        tsl = slice(split * 512, (split + 1) * 512)
        x1, x2 = pres[split][2 * pair], pres[split][2 * pair + 1]
        r1, r2 = rot[2 * pair][:, tsl], rot[2 * pair + 1][:, tsl]
        if chunk == 0:
            rope_tmp[(split, pair)] = [
                tmp.tile([128, 512], bf16, tag=t, name=t)
                for t in ("t1", "t2", "t3", "t4")]
        t1, t2, t3, t4 = rope_tmp[(split, pair)]
        if chunk == 0:
            eng.tensor_mul(t1, x1, cos_sb[:, tsl])
            eng.tensor_mul(t2, x2, sin_sb[:, tsl])
        elif chunk == 1:
            eng.tensor_sub(r1, t1, t2)
            eng.tensor_mul(t3, x2, cos_sb[:, tsl])
        else:
            eng.tensor_mul(t4, x1, sin_sb[:, tsl])
            eng.tensor_add(r2, t3, t4)

    def rope(split, pair, eng):
        """rot1 = x1*cos - x2*sin ; rot2 = x2*cos + x1*sin for one pair."""
        for chunk in range(3):
            rope_chunk(split, pair, eng, chunk)

    def repack(t0, tlen):
        """Repack a token range of rot into head-major qhT/khT; q-side DMAs
        issue from the SP queue, k-side from GpSimd (SWDGE) so the rope-gated
        issues never block the ACT compute stream."""
        tsl = slice(t0, t0 + tlen)
        for hl in range(HEADS_PER_CORE):
            d0 = hl * T + t0
            for half in range(2):
                nc.sync.dma_start(
                    out=qhT[half * 32:(half + 1) * 32, d0:d0 + tlen],
                    in_=rot[half][hl * 32:(hl + 1) * 32, tsl],
                )
                nc.gpsimd.dma_start(
                    out=khT[half * 32:(half + 1) * 32, d0:d0 + tlen],
                    in_=rot[2 + half][hl * 32:(hl + 1) * 32, tsl],
                )

    # ---- software-pipelined attention ----
    st = {}  # qt -> {p:{hl: tile}, osb:, asb:}
    ysbs = {}  # qt-pair -> ysb tile

    def wincfg(qt):
        nkt = min(qt + 1, 3)
        return nkt, max(qt - 2, 0)

    def slot_of(qt, a):
        # score block column slots: [mid, first, diag] for qt>=2 so the two
        # masked blocks (first, diag) are one contiguous [128, 256] region
        if qt < 2:
            return a
        return (1, 0, 2)[a]

    def emit_v(qt):
        """V tile for qt in [k-part, (head, 65)] layout (ones col fused)."""
        s, off = qt // 4, (qt % 4) * 128
        acc = pmm.tile([128, FQ], f32, tag="mm")
        for kc in range(8):
            nc.tensor.matmul(
                acc,
                lhsT=xT_sb[:, s * 4096 + kc * 512 + off:s * 4096 + kc * 512 + off + 128],
                rhs=wT_sb.rearrange("p (b k j) -> p b k j", b=6, k=8)[:, 4:6, kc, :],
                start=(kc == 0),
                stop=(kc == 7),
            )
        nc.scalar.copy(
            v_sb[:, qt * VROW:(qt + 1) * VROW]
            .rearrange("p (h c) -> p h c", h=HEADS_PER_CORE)[:, :, 0:DH],
            acc.rearrange("p (h d) -> p h d", h=HEADS_PER_CORE),
        )

    def emit_scores(qt, pair):
        """Transposed scores + exp + band mask for one pair of heads.
        P^T for both heads lives in one [128, 768] tile so the mask is a
        single strided DVE multiply."""
        nkt, kt0 = wincfg(qt)
        ss = st.setdefault(qt, {"p": {}})
        pp = work.tile([128, 768], bf16, tag="p")
        for i, hl in enumerate((2 * pair, 2 * pair + 1)):
            s = ps.tile([128, 384], f32, tag="s")
            for a in range(nkt):
                kt = kt0 + a
                nc.tensor.matmul(
                    s[:, slot_of(qt, a) * 128:(slot_of(qt, a) + 1) * 128],
                    lhsT=khT[:, hl * T + kt * 128:hl * T + (kt + 1) * 128],
                    rhs=qhT[:, hl * T + qt * 128:hl * T + (qt + 1) * 128],
                    start=True,
                    stop=True,
                )
            w = 128 * nkt
            nc.scalar.activation(pp[:, i * 384:i * 384 + w], s[:, :w], Exp)
            ss["p"][hl] = pp[:, i * 384:(i + 1) * 384]
        ppv = pp.rearrange("p (i c) -> p i c", i=2)
        if qt >= 2:  # mask blocks [first|diag] at cols 128:384 of each half
            nc.vector.tensor_tensor(
                ppv[:, :, 128:384], ppv[:, :, 128:384],
                amask_sb.rearrange("p (one c) -> p one c", one=1)
                .broadcast_to([128, 2, 256]),
                Mult)
        else:  # single diag block: slot nkt-1
            c0 = (nkt - 1) * 128
            nc.vector.tensor_tensor(
                ppv[:, :, c0:c0 + 128], ppv[:, :, c0:c0 + 128],
                amask_sb.rearrange("p (one c) -> p one c", one=1)
                [:, :, 128:256].broadcast_to([128, 2, 128]),
                Mult)

    def emit_ot(qt):
        """P^T @ [V|1] per head into one shared PSUM bank, then one batched
        reciprocal + one broadcast multiply for the normalization."""
        nkt, kt0 = wincfg(qt)
        ss = st[qt]
        osb = osbp.tile([128, FQ], bf16, tag="osb")
        ss["osb"] = osb
        oall = po.tile([128, VROW], f32, tag="o")
        for hl in range(HEADS_PER_CORE):
            p = ss["p"][hl]
            for a in range(nkt):
                kt = kt0 + a
                sl = slot_of(qt, a)
                nc.tensor.matmul(
                    oall[:, hl * VW:(hl + 1) * VW],
                    lhsT=p[:, sl * 128:(sl + 1) * 128],
                    rhs=v_sb[:, kt * VROW + hl * VW:kt * VROW + (hl + 1) * VW],
                    start=(a == 0),
                    stop=(a == nkt - 1),
                )
        ov = oall.rearrange("p (h c) -> p h c", c=VW)
        rc = small.tile([128, HEADS_PER_CORE], f32, tag="rc")
        rcv = rc.rearrange("p (h one) -> p h one", one=1)
        nc.vector.reciprocal(rcv, ov[:, :, DH:DH + 1])
        nc.vector.tensor_tensor(
            osb.rearrange("p (h d) -> p h d", d=DH),
            ov[:, :, 0:DH],
            rcv.broadcast_to([128, HEADS_PER_CORE, DH]),
            Mult)

    def emit_tp(qt):
        """PE-transpose the attention output to [feature, token]."""
        if qt < 0:
            return
        ss = st[qt]
        t2 = po.tile([128, FQ], bf16, tag="o", name="t2")
        for c in range(2):
            nc.tensor.transpose(
                t2[:, c * 128:(c + 1) * 128],
                ss["osb"][:, c * 128:(c + 1) * 128], id_sb)
        asb = asbp.tile([128, FQ], bf16, tag="asb")
        nc.vector.tensor_copy(asb, t2)
        ss["asb"] = asb

    def emit_outproj(qt):
        """Out-proj for qt, staged through SBUF (cast to bf16); token-tile
        pairs are stored with a single [128, 2048] DMA."""
        if qt < 0:
            return
        asb = st[qt]["asb"]
        single = qt >= QT - 2  # last two tiles stored alone (shorter tail)
        if qt % 2 == 0 or single:
            ysbs[qt // 2] = ysbp.tile([128, 2 * C], bf16, tag="ysb",
                                      name="ysb")
        ysb = ysbs[qt // 2]
        base = 0 if single else (qt % 2) * C
        for nh in range(2):
            acc = pout.tile([128, 512], f32, tag="yp")
            for kc in range(2):
                nc.tensor.matmul(
                    acc,
                    lhsT=asb[:, kc * 128:(kc + 1) * 128],
                    rhs=woT_sb[:, kc * C + nh * 512:kc * C + (nh + 1) * 512],
                    start=(kc == 0),
                    stop=(kc == 1),
                )
            if nh == 0:
                nc.scalar.copy(ysb[:, base:base + 512], acc)
            else:
                nc.vector.tensor_copy(ysb[:, base + 512:base + 1024], acc)
        if single:
            nc.sync.dma_start(
                out=y[:, qt * C:(qt + 1) * C], in_=ysb[:, 0:C])
        elif qt % 2 == 1:
            nc.sync.dma_start(
                out=y[:, (qt - 1) * C:(qt + 1) * C], in_=ysb)
        del st[qt]

    def attn_iter(qt):
        # scores/exp/mask for qt+1 are emitted one full iteration before
        # PV(qt+1) consumes them, so PV never waits on the exp->mask chain
        emit_tp(qt - 1)
        if qt + 2 < QT:
            emit_v(qt + 2)
        if qt + 1 < QT:
            emit_scores(qt + 1, 0)
            emit_scores(qt + 1, 1)
        emit_ot(qt)
        emit_outproj(qt - 1)

    # ---- prologue: projections for token half 0, RoPE on idle DVE ----
    qkv_half(0, 0)
    qkv_half(0, 1)
    rope(0, 0, nc.vector)
    rope(0, 1, nc.vector)
    repack(0, 512)  # unblocks qt 0-3 without waiting on split-1 rope
    qkv_half(1, 0)
    qkv_half(1, 1)
    rope(1, 0, nc.vector)
    rope(1, 1, nc.vector)
    repack(512, 512)
    emit_v(0)
    emit_v(1)
    qkv_half(2, 0)  # keeps the PE busy while the repack lands
    qkv_half(2, 1)
    emit_scores(0, 0)
    emit_scores(0, 1)

    # ---- attention pipeline, with split 2/3 projections interleaved ----
    # steady-state RoPE is split between GpSimd and DVE so that both the
    # q-side (rot0/1) and k-side (rot2/3) of each late repack range are
    # ready well before the score matmuls that consume them
    attn_iter(0)
    qkv_half(3, 0, alt_pre=nc.vector)
    rope(2, 1, nc.gpsimd)  # k-side ropes run first on GpSimd
    attn_iter(1)
    qkv_half(3, 1, alt_pre=nc.vector)
    rope_chunk(2, 0, nc.vector, 0)  # q-side ropes on DVE, spread out
    attn_iter(2)
    rope_chunk(2, 0, nc.vector, 1)
    rope(3, 1, nc.gpsimd)
    attn_iter(3)
    rope_chunk(2, 0, nc.vector, 2)
    repack(1024, 512)
    attn_iter(4)
    rope_chunk(3, 0, nc.vector, 0)
    attn_iter(5)
    rope_chunk(3, 0, nc.vector, 1)
    attn_iter(6)
    rope_chunk(3, 0, nc.vector, 2)
    repack(1536, 512)
    for qt in range(7, QT):
        attn_iter(qt)
    emit_tp(QT - 1)
    emit_outproj(QT - 1)


def _build_program():
    import concourse.tile as tile
    from concourse import bacc, mybir

    bf16 = mybir.dt.bfloat16

    nc = bacc.Bacc("TRN2", target_bir_lowering=False, debug=False,
                   num_devices=N_CORES)
    aps = {
        "xT": nc.dram_tensor("xT", [128, 4 * 8 * 512], bf16, kind="ExternalInput").ap(),
        "wT": nc.dram_tensor("wT", [128, 6 * 8 * 128], bf16, kind="ExternalInput").ap(),
        "woT": nc.dram_tensor("woT", [128, 2 * C], bf16, kind="ExternalInput").ap(),
        "cos4": nc.dram_tensor("cos4", [128, T], bf16, kind="ExternalInput").ap(),
        "sin4": nc.dram_tensor("sin4", [128, T], bf16, kind="ExternalInput").ap(),
        "amask01": nc.dram_tensor("amask01", [128, 256], bf16, kind="ExternalInput").ap(),
        "ident": nc.dram_tensor("ident", [128, 128], bf16, kind="ExternalInput").ap(),
        "y": nc.dram_tensor("y", [128, QT * C], bf16, kind="ExternalOutput").ap(),
    }
    from contextlib import ExitStack

    with tile.TileContext(nc) as tc, ExitStack() as ctx:
        _emit(nc, tc, aps, ctx)
    nc.compile()
    return nc


def _get_program():
    global _PROGRAM
    if _PROGRAM is None:
        _PROGRAM = _build_program()
    return _PROGRAM


def _host_inputs(x, w_qkv, w_out):
    import ml_dtypes

    bf16 = ml_dtypes.bfloat16
    x = np.asarray(x, np.float32)
    w_qkv = np.asarray(w_qkv, np.float32)
    w_out = np.asarray(w_out, np.float32)

    wq, wk, wv = w_qkv[0:C], w_qkv[C:2 * C], w_qkv[2 * C:3 * C]
    scale = 1.0 / math.sqrt(DH)

    # RoPE tables (transposed, tiled over the 4 heads of a block)
    inv_freq = 1.0 / (10000.0 ** (np.arange(0, DH, 2, dtype=np.float32) / DH))
    freqs = np.outer(np.arange(T, dtype=np.float32), inv_freq)  # [T, 32]
    cos4 = np.ascontiguousarray(np.tile(np.cos(freqs).T, (4, 1))).astype(bf16)
    sin4 = np.ascontiguousarray(np.tile(np.sin(freqs).T, (4, 1))).astype(bf16)

    # multiplicative 0/1 band masks for TRANSPOSED probabilities pT[k, q]:
    # [block kt=qt-2: allowed qq < kk | block kt=qt: allowed qq >= kk]
    i = np.arange(128)[:, None]  # kk (partitions)
    c = np.arange(128)[None, :]  # qq (free)
    m_first = (c < i).astype(np.float32)
    m_last = (c >= i).astype(np.float32)
    amask01 = np.ascontiguousarray(
        np.concatenate([m_first, m_last], axis=1)).astype(bf16)
    ident = np.eye(128, dtype=np.float32).astype(bf16)

    # x relayout to [C-chunk partition, (split, kc, 512)], matching SBUF
    xT = []
    for b in range(B):
        a = x[b].reshape(4, 512, 8, 128)  # [s, t', kc, p]
        xT.append(np.ascontiguousarray(
            a.transpose(3, 0, 2, 1).reshape(128, 4 * 8 * 512)).astype(bf16))

    in_maps = []
    for core in range(N_CORES):
        b, g = divmod(core, 4)
        hs = range(4 * g, 4 * g + 4)
        rows = []
        for half in range(2):  # q_x1, q_x2
            rows.append(np.concatenate(
                [wq[h * DH + 32 * half:h * DH + 32 * half + 32] for h in hs]) * scale)
        for half in range(2):  # k_x1, k_x2
            rows.append(np.concatenate(
                [wk[h * DH + 32 * half:h * DH + 32 * half + 32] for h in hs]))
        rows.append(wv[g * FQ:(g + 1) * FQ])
        wmat = np.concatenate(rows)  # [768, C]
        # relayout to [C-chunk partition, (blk, kc, 128 rows)]
        wa = wmat.reshape(6, 128, 8, 128)  # [blk, j, kc, p]
        wT = np.ascontiguousarray(
            wa.transpose(3, 0, 2, 1).reshape(128, 6 * 8 * 128)).astype(bf16)
        # w_out columns for this head group, [feat-chunk part, (kc, C)]
        wo = w_out[:, g * FQ:(g + 1) * FQ].T.reshape(2, 128, C)  # [kc, p, e]
        woT = np.ascontiguousarray(
            wo.transpose(1, 0, 2).reshape(128, 2 * C)).astype(bf16)
        in_maps.append({
            "xT": xT[b], "wT": wT, "woT": woT,
            "cos4": cos4, "sin4": sin4, "amask01": amask01, "ident": ident,
        })
    return in_maps


def kernel(x, w_qkv, w_out, _trace=False):
    from concourse import bass_utils

    nc = _get_program()
    in_maps = _host_inputs(x, w_qkv, w_out)
    res = bass_utils.run_bass_kernel_spmd(
        nc, in_maps, core_ids=list(range(N_CORES)), trace=_trace,
    )
    parts = []
    for core in range(N_CORES):
        yv = np.asarray(res.results[core]["y"], dtype=np.float32)
        # [128, (qt, C)] -> [T, C]
        parts.append(yv.reshape(128, QT, C).transpose(1, 0, 2).reshape(T, C))
    out = np.stack([
        parts[0] + parts[1] + parts[2] + parts[3],
        parts[4] + parts[5] + parts[6] + parts[7],
    ])
    if _trace:
        return out, res
    return out


# revision 46
# speedup vs baseline: 1.0490x; 1.0306x over previous
"""Sliding-window causal self-attention (B=2, T=2048, C=1024, H=16, Dh=64,
window=256) + QKV/out projections, sharded over 8 NeuronCores as
data-parallel over B (2) x tensor-parallel over head groups (4 heads/core).

Layout strategy ("sT scheme"): scores are computed TRANSPOSED
(sT[k, q] = khT^T @ qhT) so the exp() activation writes P^T straight to
SBUF. The band mask is a post-exp 0/1 multiply on bf16 SBUF data; score
blocks are stored [mid, first, diag] so the two masked blocks are one
contiguous [128, 256] region and the mask is a single DVE multiply per
head pair. Row sums come free from a ones-column appended to each head's
V tile; the 4 per-head PV outputs share one PSUM bank so normalization
is one batched reciprocal + one broadcast multiply per query tile. The
attention output is transposed back with PE identity matmuls before the
out-proj.

DMA strategy: all DRAM tensors are pre-arranged on the host to exactly
match their SBUF destination layout, so every load is a single dma_start
with one max-length descriptor per partition (sequencer issue cost and
queue descriptor count are the real DMA bottlenecks, not bytes). The
first QKV chain's inputs (w q-blocks, x split 0) are the first issues on
the two HWDGE queues; everything else streams behind in need-order.
Output tiles are stored as [128, 2048] pairs (8 stores total).

Scheduling: software-pipelined per query tile as in the previous
revision: QKV token-splits are emitted ahead of their attention
consumers; RoPE runs on DVE (splits 0-1) and the otherwise-idle GpSimd
engine (splits 2-3); q-side repack DMAs issue from the SP queue and
k-side from the ACT queue.
"""

import math

import numpy as np

B = 2
T = 2048
C = 1024
H = 16
DH = 64
WINDOW = 256
HEADS_PER_CORE = 4
N_CORES = 8
QT = T // 128  # 16 query tiles of 128
FQ = HEADS_PER_CORE * DH  # 256 local features
VW = DH + 1  # per-head v columns incl the fused ones column
VROW = HEADS_PER_CORE * VW  # 260 v columns per key tile

_PROGRAM = None  # compile once per process


def _emit(nc, tc, aps, ctx):
    from concourse import mybir

    f32 = mybir.dt.float32
    bf16 = mybir.dt.bfloat16
    Exp = mybir.ActivationFunctionType.Exp
    Mult = mybir.AluOpType.mult

    xT, wT, woT, cos4, sin4, amask01, ident, y = (
        aps["xT"], aps["wT"], aps["woT"], aps["cos4"], aps["sin4"],
        aps["amask01"], aps["ident"], aps["y"],
    )

    consts = ctx.enter_context(tc.tile_pool(name="consts", bufs=1))
    stage = ctx.enter_context(tc.tile_pool(name="stage", bufs=1))
    pre = ctx.enter_context(tc.tile_pool(name="pre", bufs=8))
    tmp = ctx.enter_context(tc.tile_pool(name="tmp", bufs=3))
    work = ctx.enter_context(tc.tile_pool(name="work", bufs=6))
    osbp = ctx.enter_context(tc.tile_pool(name="osbp", bufs=2))
    asbp = ctx.enter_context(tc.tile_pool(name="asbp", bufs=2))
    ysbp = ctx.enter_context(tc.tile_pool(name="ysbp", bufs=3))
    small = ctx.enter_context(tc.tile_pool(name="small", bufs=4))
    pmm = ctx.enter_context(tc.tile_pool(name="pmm", bufs=2, space="PSUM"))
    pout = ctx.enter_context(tc.tile_pool(name="pout", bufs=2, space="PSUM"))
    ps = ctx.enter_context(tc.tile_pool(name="ps", bufs=2, space="PSUM"))
    po = ctx.enter_context(tc.tile_pool(name="po", bufs=2, space="PSUM"))

    # ---- resident inputs ----
    # x in [C-chunk partition, (split, kc, 512 tokens)] — matches DRAM
    xT_sb = consts.tile([128, 4 * 8 * 512], bf16, tag="xT")
    # w in [C-chunk partition, (blk, kc, 128 rows)], blk = q1 q2 k1 k2 v0 v1
    wT_sb = consts.tile([128, 6 * 8 * 128], bf16, tag="wT")
    woT_sb = consts.tile([128, 2 * C], bf16, tag="woT")
    cos_sb = consts.tile([128, T], bf16, tag="cos")
    sin_sb = consts.tile([128, T], bf16, tag="sin")
    amask_sb = consts.tile([128, 256], bf16, tag="amask")
    id_sb = consts.tile([128, 128], bf16, tag="ident")

    # SP queue: first-chain inputs first, then stream in need-order.
    # Each x split is ONE dma so a QKV chain waits once and then runs all
    # 8 accumulation steps gapless (mid-chain stalls also reset the PE
    # clock ramp).
    nc.sync.dma_start(out=wT_sb[:, 0:2048], in_=wT[:, 0:2048])  # q1 q2
    for s in range(2):
        nc.sync.dma_start(out=xT_sb[:, s * 4096:(s + 1) * 4096],
                          in_=xT[:, s * 4096:(s + 1) * 4096])
    nc.sync.dma_start(out=cos_sb, in_=cos4)
    nc.sync.dma_start(out=sin_sb, in_=sin4)
    # ACT queue: k/v weights, mask, x splits 2-3, transpose id, out-proj w.
    # amask goes first so the big w load doesn't race x split 0 for HBM.
    nc.scalar.dma_start(out=amask_sb, in_=amask01)
    nc.scalar.dma_start(out=wT_sb[:, 2048:6144], in_=wT[:, 2048:6144])
    for s in range(2, 4):
        nc.scalar.dma_start(out=xT_sb[:, s * 4096:(s + 1) * 4096],
                            in_=xT[:, s * 4096:(s + 1) * 4096])
    nc.scalar.dma_start(out=id_sb, in_=ident)
    nc.scalar.dma_start(out=woT_sb, in_=woT)

    # ---- persistent intermediates ----
    # rotated q/k blocks [q_x1, q_x2, k_x1, k_x2], each [128=(4h x 32d), T]
    rot = [stage.tile([128, T], bf16, tag=f"rot{i}", name=f"rot{i}")
           for i in range(4)]
    qhT = stage.tile([64, HEADS_PER_CORE * T], bf16, tag="qhT")
    khT = stage.tile([64, HEADS_PER_CORE * T], bf16, tag="khT")
    # v in [k-token-part, (kt, head, 65)] layout; col 64 of each head = ones
    v_sb = stage.tile([128, QT * VROW], bf16, tag="v")
    nc.gpsimd.memset(
        v_sb.rearrange("p (g c) -> p g c", c=VW)[:, :, DH:DH + 1], 1.0)

    pres = {}  # split -> [pre tiles]

    def qkv_half(split, pair, alt_pre=None):
        """QKV projection matmuls + PSUM->SBUF casts for the q or k blocks
        of one token slice. alt_pre routes the odd-block cast to another
        engine to avoid piling copies onto ACT ahead of critical exps."""
        ptiles = pres.setdefault(split, [])
        for blk in (2 * pair, 2 * pair + 1):  # q_x1 q_x2 | k_x1 k_x2
            acc = pmm.tile([128, 512], f32, tag="mm")
            for kc in range(8):
                nc.tensor.matmul(
                    acc,
                    lhsT=wT_sb[:, blk * 1024 + kc * 128:blk * 1024 + (kc + 1) * 128],
                    rhs=xT_sb[:, split * 4096 + kc * 512:split * 4096 + (kc + 1) * 512],
                    start=(kc == 0),
                    stop=(kc == 7),
                )
            pblk = pre.tile([128, 512], bf16, tag="pre", name=f"pre{split}{blk}")
            if alt_pre is not None and blk % 2 == 1:
                alt_pre.tensor_copy(pblk, acc)
            else:
                nc.scalar.copy(pblk, acc)
            ptiles.append(pblk)

    rope_tmp = {}

    def rope_chunk(split, pair, eng, chunk):
        """Two of the six rope ops; chunks can be spread across iterations
        so a DVE rope burst never delays the latency-critical mask ops."""
        tsl = slice(split * 512, (split + 1) * 512)
        x1, x2 = pres[split][2 * pair], pres[split][2 * pair + 1]
        r1, r2 = rot[2 * pair][:, tsl], rot[2 * pair + 1][:, tsl]
        if chunk == 0:
            rope_tmp[(split, pair)] = [
                tmp.tile([128, 512], bf16, tag=t, name=t)
                for t in ("t1", "t2", "t3", "t4")]
        t1, t2, t3, t4 = rope_tmp[(split, pair)]
        if chunk == 0:
            eng.tensor_mul(t1, x1, cos_sb[:, tsl])
            eng.tensor_mul(t2, x2, sin_sb[:, tsl])
        elif chunk == 1:
            eng.tensor_sub(r1, t1, t2)
            eng.tensor_mul(t3, x2, cos_sb[:, tsl])
        else:
            eng.tensor_mul(t4, x1, sin_sb[:, tsl])
            eng.tensor_add(r2, t3, t4)

    def rope(split, pair, eng):
        """rot1 = x1*cos - x2*sin ; rot2 = x2*cos + x1*sin for one pair."""
        for chunk in range(3):
            rope_chunk(split, pair, eng, chunk)

    def repack(t0, tlen):
        """Repack a token range of rot into head-major qhT/khT; q-side DMAs
        issue from the SP queue, k-side from GpSimd (SWDGE) so the rope-gated
        issues never block the ACT compute stream."""
        tsl = slice(t0, t0 + tlen)
        for hl in range(HEADS_PER_CORE):
            d0 = hl * T + t0
            for half in range(2):
                nc.sync.dma_start(
                    out=qhT[half * 32:(half + 1) * 32, d0:d0 + tlen],
                    in_=rot[half][hl * 32:(hl + 1) * 32, tsl],
                )
                nc.gpsimd.dma_start(
                    out=khT[half * 32:(half + 1) * 32, d0:d0 + tlen],
                    in_=rot[2 + half][hl * 32:(hl + 1) * 32, tsl],
                )

    # ---- software-pipelined attention ----
    st = {}  # qt -> {p:{hl: tile}, osb:, asb:}
    ysbs = {}  # qt-pair -> ysb tile

    def wincfg(qt):
        nkt = min(qt + 1, 3)
        return nkt, max(qt - 2, 0)

    def slot_of(qt, a):
        # score block column slots: [mid, first, diag] for qt>=2 so the two
        # masked blocks (first, diag) are one contiguous [128, 256] region
        if qt < 2:
            return a
        return (1, 0, 2)[a]

    def emit_v(qt):
        """V tile for qt in [k-part, (head, 65)] layout (ones col fused)."""
        s, off = qt // 4, (qt % 4) * 128
        acc = pmm.tile([128, FQ], f32, tag="mm")
        for kc in range(8):
            nc.tensor.matmul(
                acc,
                lhsT=xT_sb[:, s * 4096 + kc * 512 + off:s * 4096 + kc * 512 + off + 128],
                rhs=wT_sb.rearrange("p (b k j) -> p b k j", b=6, k=8)[:, 4:6, kc, :],
                start=(kc == 0),
                stop=(kc == 7),
            )
        nc.scalar.copy(
            v_sb[:, qt * VROW:(qt + 1) * VROW]
            .rearrange("p (h c) -> p h c", h=HEADS_PER_CORE)[:, :, 0:DH],
            acc.rearrange("p (h d) -> p h d", h=HEADS_PER_CORE),
        )

    def emit_scores(qt, pair):
        """Transposed scores + exp + band mask for one pair of heads.
        P^T for both heads lives in one [128, 768] tile so the mask is a
        single strided DVE multiply."""
        nkt, kt0 = wincfg(qt)
        ss = st.setdefault(qt, {"p": {}})
        pp = work.tile([128, 768], bf16, tag="p")
        for i, hl in enumerate((2 * pair, 2 * pair + 1)):
            s = ps.tile([128, 384], f32, tag="s")
            for a in range(nkt):
                kt = kt0 + a
                nc.tensor.matmul(
                    s[:, slot_of(qt, a) * 128:(slot_of(qt, a) + 1) * 128],
                    lhsT=khT[:, hl * T + kt * 128:hl * T + (kt + 1) * 128],
                    rhs=qhT[:, hl * T + qt * 128:hl * T + (qt + 1) * 128],
                    start=True,
                    stop=True,
                )
            w = 128 * nkt
            nc.scalar.activation(pp[:, i * 384:i * 384 + w], s[:, :w], Exp)
            ss["p"][hl] = pp[:, i * 384:(i + 1) * 384]
        ppv = pp.rearrange("p (i c) -> p i c", i=2)
        if qt >= 2:  # mask blocks [first|diag] at cols 128:384 of each half
            nc.vector.tensor_tensor(
                ppv[:, :, 128:384], ppv[:, :, 128:384],
                amask_sb.rearrange("p (one c) -> p one c", one=1)
                .broadcast_to([128, 2, 256]),
                Mult)
        else:  # single diag block: slot nkt-1
            c0 = (nkt - 1) * 128
            nc.vector.tensor_tensor(
                ppv[:, :, c0:c0 + 128], ppv[:, :, c0:c0 + 128],
                amask_sb.rearrange("p (one c) -> p one c", one=1)
                [:, :, 128:256].broadcast_to([128, 2, 128]),
                Mult)

    def emit_ot(qt):
        """P^T @ [V|1] per head into one shared PSUM bank, then one batched
        reciprocal + one broadcast multiply for the normalization."""
        nkt, kt0 = wincfg(qt)
        ss = st[qt]
        osb = osbp.tile([128, FQ], bf16, tag="osb")
        ss["osb"] = osb
        oall = po.tile([128, VROW], f32, tag="o")
        for hl in range(HEADS_PER_CORE):
            p = ss["p"][hl]
            for a in range(nkt):
                kt = kt0 + a
                sl = slot_of(qt, a)
                nc.tensor.matmul(
                    oall[:, hl * VW:(hl + 1) * VW],
                    lhsT=p[:, sl * 128:(sl + 1) * 128],
                    rhs=v_sb[:, kt * VROW + hl * VW:kt * VROW + (hl + 1) * VW],
                    start=(a == 0),
                    stop=(a == nkt - 1),
                )
        ov = oall.rearrange("p (h c) -> p h c", c=VW)
        rc = small.tile([128, HEADS_PER_CORE], f32, tag="rc")
        rcv = rc.rearrange("p (h one) -> p h one", one=1)
        nc.vector.reciprocal(rcv, ov[:, :, DH:DH + 1])
        nc.vector.tensor_tensor(
            osb.rearrange("p (h d) -> p h d", d=DH),
            ov[:, :, 0:DH],
            rcv.broadcast_to([128, HEADS_PER_CORE, DH]),
            Mult)

    def emit_tp(qt):
        """PE-transpose the attention output to [feature, token]."""
        if qt < 0:
            return
        ss = st[qt]
        t2 = po.tile([128, FQ], bf16, tag="o", name="t2")
        for c in range(2):
            nc.tensor.transpose(
                t2[:, c * 128:(c + 1) * 128],
                ss["osb"][:, c * 128:(c + 1) * 128], id_sb)
        asb = asbp.tile([128, FQ], bf16, tag="asb")
        nc.vector.tensor_copy(asb, t2)
        ss["asb"] = asb

    def emit_outproj(qt):
        """Out-proj for qt, staged through SBUF (cast to bf16); token-tile
        pairs are stored with a single [128, 2048] DMA."""
        if qt < 0:
            return
        asb = st[qt]["asb"]
        single = qt >= QT - 2  # last two tiles stored alone (shorter tail)
        if qt % 2 == 0 or single:
            ysbs[qt // 2] = ysbp.tile([128, 2 * C], bf16, tag="ysb",
                                      name="ysb")
        ysb = ysbs[qt // 2]
        base = 0 if single else (qt % 2) * C
        for nh in range(2):
            acc = pout.tile([128, 512], f32, tag="yp")
            for kc in range(2):
                nc.tensor.matmul(
                    acc,
                    lhsT=asb[:, kc * 128:(kc + 1) * 128],
                    rhs=woT_sb[:, kc * C + nh * 512:kc * C + (nh + 1) * 512],
                    start=(kc == 0),
                    stop=(kc == 1),
                )
            if nh == 0:
                nc.scalar.copy(ysb[:, base:base + 512], acc)
            else:
                nc.vector.tensor_copy(ysb[:, base + 512:base + 1024], acc)
        if single:
            nc.sync.dma_start(
                out=y[:, qt * C:(qt + 1) * C], in_=ysb[:, 0:C])
        elif qt % 2 == 1:
            nc.sync.dma_start(
                out=y[:, (qt - 1) * C:(qt + 1) * C], in_=ysb)
        del st[qt]

    def attn_iter(qt):
        # scores/exp/mask for qt+1 are emitted one full iteration before
        # PV(qt+1) consumes them, so PV never waits on the exp->mask chain
        emit_tp(qt - 1)
        if qt + 2 < QT:
            emit_v(qt + 2)
        if qt + 1 < QT:
            emit_scores(qt + 1, 0)
            emit_scores(qt + 1, 1)
        emit_ot(qt)
        emit_outproj(qt - 1)

    # ---- prologue: projections for token half 0, RoPE on idle DVE ----
    qkv_half(0, 0)
    qkv_half(0, 1)
    rope(0, 0, nc.vector)
    rope(0, 1, nc.vector)
    repack(0, 512)  # unblocks qt 0-3 without waiting on split-1 rope
    qkv_half(1, 0)
    qkv_half(1, 1)
    rope(1, 0, nc.vector)
    rope(1, 1, nc.vector)
    repack(512, 512)
    emit_v(0)
    emit_v(1)
    qkv_half(2, 0)  # keeps the PE busy while the repack lands
    qkv_half(2, 1)
    emit_scores(0, 0)
    emit_scores(0, 1)

    # ---- attention pipeline, with split 2/3 projections interleaved ----
    # steady-state RoPE is split between GpSimd and DVE so that both the
    # q-side (rot0/1) and k-side (rot2/3) of each late repack range are
    # ready well before the score matmuls that consume them
    attn_iter(0)
    qkv_half(3, 0, alt_pre=nc.vector)
    rope(2, 1, nc.gpsimd)  # k-side ropes run first on GpSimd
    attn_iter(1)
    qkv_half(3, 1, alt_pre=nc.vector)
    rope_chunk(2, 0, nc.vector, 0)  # q-side ropes on DVE, spread out
    attn_iter(2)
    rope_chunk(2, 0, nc.vector, 1)
    rope(3, 1, nc.gpsimd)
    attn_iter(3)
    rope_chunk(2, 0, nc.vector, 2)
    repack(1024, 512)
    attn_iter(4)
    rope_chunk(3, 0, nc.vector, 0)
    attn_iter(5)
    rope_chunk(3, 0, nc.vector, 1)
    attn_iter(6)
    rope_chunk(3, 0, nc.vector, 2)
    repack(1536, 512)
    for qt in range(7, QT):
        attn_iter(qt)
    emit_tp(QT - 1)
    emit_outproj(QT - 1)


def _build_program():
    import concourse.tile as tile
    from concourse import bacc, mybir

    bf16 = mybir.dt.bfloat16

    nc = bacc.Bacc("TRN2", target_bir_lowering=False, debug=False,
                   num_devices=N_CORES)
    aps = {
        "xT": nc.dram_tensor("xT", [128, 4 * 8 * 512], bf16, kind="ExternalInput").ap(),
        "wT": nc.dram_tensor("wT", [128, 6 * 8 * 128], bf16, kind="ExternalInput").ap(),
        "woT": nc.dram_tensor("woT", [128, 2 * C], bf16, kind="ExternalInput").ap(),
        "cos4": nc.dram_tensor("cos4", [128, T], bf16, kind="ExternalInput").ap(),
        "sin4": nc.dram_tensor("sin4", [128, T], bf16, kind="ExternalInput").ap(),
        "amask01": nc.dram_tensor("amask01", [128, 256], bf16, kind="ExternalInput").ap(),
        "ident": nc.dram_tensor("ident", [128, 128], bf16, kind="ExternalInput").ap(),
        "y": nc.dram_tensor("y", [128, QT * C], bf16, kind="ExternalOutput").ap(),
    }
    from contextlib import ExitStack

    with tile.TileContext(nc) as tc, ExitStack() as ctx:
        _emit(nc, tc, aps, ctx)
    nc.compile()
    return nc


def _get_program():
    global _PROGRAM
    if _PROGRAM is None:
        _PROGRAM = _build_program()
    return _PROGRAM


def _host_inputs(x, w_qkv, w_out):
    import ml_dtypes

    bf16 = ml_dtypes.bfloat16
    x = np.asarray(x, np.float32)
    w_qkv = np.asarray(w_qkv, np.float32)
    w_out = np.asarray(w_out, np.float32)

    wq, wk, wv = w_qkv[0:C], w_qkv[C:2 * C], w_qkv[2 * C:3 * C]
    scale = 1.0 / math.sqrt(DH)

    # RoPE tables (transposed, tiled over the 4 heads of a block)
    inv_freq = 1.0 / (10000.0 ** (np.arange(0, DH, 2, dtype=np.float32) / DH))
    freqs = np.outer(np.arange(T, dtype=np.float32), inv_freq)  # [T, 32]
    cos4 = np.ascontiguousarray(np.tile(np.cos(freqs).T, (4, 1))).astype(bf16)
    sin4 = np.ascontiguousarray(np.tile(np.sin(freqs).T, (4, 1))).astype(bf16)

    # multiplicative 0/1 band masks for TRANSPOSED probabilities pT[k, q]:
    # [block kt=qt-2: allowed qq < kk | block kt=qt: allowed qq >= kk]
    i = np.arange(128)[:, None]  # kk (partitions)
    c = np.arange(128)[None, :]  # qq (free)
    m_first = (c < i).astype(np.float32)
    m_last = (c >= i).astype(np.float32)
    amask01 = np.ascontiguousarray(
        np.concatenate([m_first, m_last], axis=1)).astype(bf16)
    ident = np.eye(128, dtype=np.float32).astype(bf16)

    # x relayout to [C-chunk partition, (split, kc, 512)], matching SBUF
    xT = []
    for b in range(B):
        a = x[b].reshape(4, 512, 8, 128)  # [s, t', kc, p]
        xT.append(np.ascontiguousarray(
            a.transpose(3, 0, 2, 1).reshape(128, 4 * 8 * 512)).astype(bf16))

    in_maps = []
    for core in range(N_CORES):
        b, g = divmod(core, 4)
        hs = range(4 * g, 4 * g + 4)
        rows = []
        for half in range(2):  # q_x1, q_x2
            rows.append(np.concatenate(
                [wq[h * DH + 32 * half:h * DH + 32 * half + 32] for h in hs]) * scale)
        for half in range(2):  # k_x1, k_x2
            rows.append(np.concatenate(
                [wk[h * DH + 32 * half:h * DH + 32 * half + 32] for h in hs]))
        rows.append(wv[g * FQ:(g + 1) * FQ])
        wmat = np.concatenate(rows)  # [768, C]
        # relayout to [C-chunk partition, (blk, kc, 128 rows)]
        wa = wmat.reshape(6, 128, 8, 128)  # [blk, j, kc, p]
        wT = np.ascontiguousarray(
            wa.transpose(3, 0, 2, 1).reshape(128, 6 * 8 * 128)).astype(bf16)
        # w_out columns for this head group, [feat-chunk part, (kc, C)]
        wo = w_out[:, g * FQ:(g + 1) * FQ].T.reshape(2, 128, C)  # [kc, p, e]
        woT = np.ascontiguousarray(
            wo.transpose(1, 0, 2).reshape(128, 2 * C)).astype(bf16)
        in_maps.append({
            "xT": xT[b], "wT": wT, "woT": woT,
            "cos4": cos4, "sin4": sin4, "amask01": amask01, "ident": ident,
        })
    return in_maps


def kernel(x, w_qkv, w_out, _trace=False):
    from concourse import bass_utils

    nc = _get_program()
    in_maps = _host_inputs(x, w_qkv, w_out)
    res = bass_utils.run_bass_kernel_spmd(
        nc, in_maps, core_ids=list(range(N_CORES)), trace=_trace,
    )
    parts = []
    for core in range(N_CORES):
        yv = np.asarray(res.results[core]["y"], dtype=np.float32)
        # [128, (qt, C)] -> [T, C]
        parts.append(yv.reshape(128, QT, C).transpose(1, 0, 2).reshape(T, C))
    out = np.stack([
        parts[0] + parts[1] + parts[2] + parts[3],
        parts[4] + parts[5] + parts[6] + parts[7],
    ])
    if _trace:
        return out, res
    return out
